# revision 30
# baseline (speedup 1.0000x reference)
"""BEV distillation mask generator (CenterPoint-style gaussian max-scatter) on TRN2.

Strategy (8 NeuronCores, data-parallel):
  core c handles frame c//2, box-half c%2 (1280 of 2560 boxes per frame).
  Per core:
    1. per-box params (radius bucket, cell, value) via DVE ops, boxes on
       partitions ([128, 10] tiles); radius via r_i = a_i*b + sqrt(beta_i*b^2
       + gamma_i*wh) with host-sent coefficient rows (one ACT sqrt)
    2. exponential encoding u ~= 2^(K*v) via the inverse float-bits trick
       (bits = (K*v+127)*2^23): f32 PSUM *sum* over colliding boxes
       approximates *max* with error <= log2(n)/K; the data has only 2-way
       collisions -> error <= 1/120 ~ 0.008 << 2e-2 tol.
    3. scatter u to 8 per-bucket point images (descending buckets 9..2,
       block index 9-b) via one-hot matmuls on PE
    4. decode: v-hat = log2(max(S,1.1))/K via the same float-bits map
       (exact inverse for singletons), Ln(v-hat) on ACT, per-bucket
       2*sigma^2 scale -> scaled-log image, fp16
    5. gaussian max-envelope = separable 2-pass shift-max DT in scaled-log
       space (additive -mag^2, bucket-independent, one merged 1024-wide
       chain): bias planes on DVE/ACT/Pool by consumption time, serial
       fp16 tensor_tensor maxes (2x) on DVE; per-block transposes (pass 1)
       and heat exps (pass 2) fire as blocks finish
    6. pairwise bucket max -> [x, 2*128] fp16 out
  Host combines column blocks, box-halves and frames with np.maximum and
  transposes to [y, x] (max-scatter is commutative) -> [4,1,128,128] f32.
"""
import numpy as np

FEAT = 128
NBOX = 1280          # boxes per core (half frame)
NT = NBOX // 128     # 10 box tiles
BUCKETS = [9, 8, 7, 6, 5, 4, 3, 2]   # block j holds bucket 9-j
K_ENC = 120.0
LN_BIAS = 1.1
LNEXP_SET_ID = 6     # act_info.json index of natural_log_exp_and_others
# radius: r_i = alpha_i*b1 + sqrt(beta_i*b1^2 + gamma_i*w*h), b1 = h+w (fm)
R_ALPHA = (0.5, 1.0, -0.1)
R_BETA = (0.25, 1.0, 0.01)
R_GAMMA = (-0.25 * 4.0 * 0.9 / 1.1, -3.6, 0.36)

_prog_cache = {}


def _f(x):
    return float(np.float32(x))


def _build_program():
    import concourse.bass as bass
    import concourse.tile as tile
    from concourse import bacc, mybir

    dt = mybir.dt
    Alu = mybir.AluOpType
    Act = mybir.ActivationFunctionType

    nc = bacc.Bacc("TRN2", target_bir_lowering=False, debug=False, num_devices=8)

    NPAR = 6 * NT + 9 * NT
    par_d = nc.dram_tensor("par", [128, NPAR], dt.float32, kind="ExternalInput").ap()
    cst_d = nc.dram_tensor("cst", [128, 1152], dt.float16, kind="ExternalInput").ap()
    hm_d = nc.dram_tensor("hm", [128, 1024], dt.float16, kind="ExternalOutput").ap()

    RECIP08 = _f(1.0 / np.float64(np.float32(0.8)))
    MAGIC = _f(8388608.0)

    def xp3(ap, dims, extra_off=0):
        return type(ap)(ap.tensor, ap.offset + extra_off, [ap.ap[0]] + dims)

    with tile.TileContext(nc) as tc:
        with (
            tc.tile_pool(name="const", bufs=1) as cpool,
            tc.tile_pool(name="par", bufs=1) as ppool,
            tc.tile_pool(name="mm", bufs=NT) as mpool,
            tc.tile_pool(name="dtw", bufs=1) as wpool,
        ):
            par = ppool.tile([128, NPAR], dt.float32, name="par")
            nc.sync.dma_start(par[:], par_d)
            cst = cpool.tile([128, 1152], dt.float16, name="cst")
            nc.scalar.dma_start(cst[:, 1024:1152], cst_d[:, 1024:1152])
            nc.sync.dma_start(cst[:, 0:1024], cst_d[:, 0:1024])
            x = par[:, 0:NT]
            y = par[:, NT:2 * NT]
            wl = par[:, 2 * NT:4 * NT]
            sc = par[:, 4 * NT:5 * NT]
            cl = par[:, 5 * NT:6 * NT]
            arow = par[:, 6 * NT:9 * NT]
            brow = par[:, 9 * NT:12 * NT]
            grow = par[:, 12 * NT:15 * NT]
            iota1024 = cst[:, 0:1024]
            iota128 = cst[:, 0:128]
            ident16 = cst[:, 1024:1152]

            V = nc.vector    # DVE
            A = nc.scalar    # ACT
            G = nc.gpsimd    # Pool
            PE = nc.tensor

            _ptn = [0]

            def pt(shape=(128, NT), dtt=None, pool=ppool):
                _ptn[0] += 1
                return pool.tile(list(shape), dtt or dt.float32, name=f"pt{_ptn[0]}")

            def floor_(dst, src_ap, scr):
                V.tensor_scalar(dst, src_ap, MAGIC, MAGIC, Alu.add, Alu.subtract)
                V.tensor_tensor(scr, dst, src_ap, Alu.is_gt)
                V.tensor_tensor(dst, dst, scr, Alu.subtract)

            def b3(t):   # broadcast a [128, NT] view over the 3 radius formulas
                return xp3(t, [[0, 3], [1, NT]])

            # ---- radius (part 1: up to the sqrt argument) ----
            wlf = pt((128, 2 * NT))
            V.tensor_scalar(wlf[:], wl, RECIP08, None, Alu.mult)
            w_fm, l_fm = wlf[:, 0:NT], wlf[:, NT:2 * NT]
            b1 = pt()
            V.tensor_tensor(b1[:], l_fm, w_fm, Alu.add)
            twh = pt()
            V.tensor_tensor(twh[:], w_fm, l_fm, Alu.mult)
            b1sq = pt()
            V.tensor_tensor(b1sq[:], b1[:], b1[:], Alu.mult)
            D3 = pt((128, 3 * NT))
            V.tensor_tensor(D3[:], b3(b1sq[:]), brow, Alu.mult)
            G3 = pt((128, 3 * NT))
            V.tensor_tensor(G3[:], b3(twh[:]), grow, Alu.mult)
            V.tensor_tensor(D3[:], D3[:], G3[:], Alu.add)
            A.activation(D3[:], D3[:], Act.Sqrt)

            # ---- value + encode (fills the sqrt round-trip) ----
            ne7 = pt(); V.tensor_scalar(ne7[:], cl, _f(7.0), None, Alu.not_equal)
            sm = pt(); V.scalar_tensor_tensor(sm[:], cl, _f(5.0), ne7[:], Alu.is_ge, Alu.mult)
            vm = pt(); V.tensor_tensor(vm[:], sm[:], sc, Alu.mult)
            V.tensor_scalar(vm[:], vm[:], _f(0.5), _f(0.5), Alu.mult, Alu.add)
            mgf = pt(); V.tensor_scalar(mgf[:], cl, _f(0.0), None, Alu.is_ge)
            dv = pt()
            V.tensor_tensor(dv[:], vm[:], sc, Alu.subtract)
            V.tensor_tensor(dv[:], dv[:], mgf[:], Alu.mult)
            v = pt()
            V.tensor_tensor(v[:], sc, dv[:], Alu.add)
            qf = pt()
            V.tensor_scalar(qf[:], v[:], _f(K_ENC * 8388608.0),
                            _f(127.0 * 8388608.0), Alu.mult, Alu.add)
            qi = pt(dtt=dt.int32)
            V.tensor_copy(qi[:], qf[:])
            u = qi[:].bitcast(dt.float32)

            # ---- centers; Pool builds the y one-hots off cy ----
            txv = pt(); V.tensor_scalar(txv[:], x, _f(51.2), RECIP08, Alu.add, Alu.mult)
            tyv = pt(); V.tensor_scalar(tyv[:], y, _f(51.2), RECIP08, Alu.add, Alu.mult)
            fscr = pt()
            cx = pt(); floor_(cx[:], txv[:], fscr[:])
            cy = pt(); floor_(cy[:], tyv[:], fscr[:])
            eys = []
            for t in range(NT):
                ey_t = mpool.tile([128, 128], dt.bfloat16, name="ey")
                G.tensor_scalar(ey_t[:], iota128, cy[:, t:t + 1], None,
                                Alu.is_equal)
                eys.append(ey_t)

            # ---- radius part 2 + bucket code ----
            R3 = pt((128, 3 * NT))
            V.tensor_tensor(R3[:], b3(b1[:]), arow, Alu.mult)
            V.tensor_tensor(R3[:], R3[:], D3[:], Alu.add)
            r = pt()
            V.tensor_tensor(r[:], R3[:, 0:NT], R3[:, NT:2 * NT], Alu.min)
            V.tensor_tensor(r[:], r[:], R3[:, 2 * NT:3 * NT], Alu.min)
            fb = pt(); floor_(fb[:], r[:], fscr[:])
            jc = pt()
            V.tensor_scalar(jc[:], fb[:], _f(2.0), _f(9.0), Alu.max, Alu.min)
            V.tensor_scalar(jc[:], jc[:], _f(-128.0), _f(1152.0), Alu.mult, Alu.add)
            cstar = pt()
            V.tensor_tensor(cstar[:], jc[:], cx[:], Alu.add)

            lnbias = cpool.tile([128, 1], dt.float32, name="lnbias")
            V.memset(lnbias[:], _f(-127.0 / K_ENC))

            # ---- one-hots + scatter matmuls ----
            with (
                tc.tile_pool(name="psS", bufs=2, space="PSUM") as psS,
                tc.tile_pool(name="psT", bufs=1, space="PSUM") as psT,
            ):
                S = [psS.tile([128, 512], dt.float32, name=f"S{h}") for h in range(2)]
                # p-state warmup: keep PE busy from early on so the real
                # matmul stream runs at full clock
                trash = psT.tile([128, 128], dt.float16, name="trash")
                for _ in range(17):
                    PE.transpose(trash[:], ident16, ident16)
                tiles = []
                for t in range(NT):
                    rhs_t = mpool.tile([128, 1024], dt.float16, name="rhs")
                    V.tensor_scalar(rhs_t[:, 512:1024], iota1024[:, 512:1024],
                                    cstar[:, t:t + 1], None, Alu.is_equal)
                    lh_t = mpool.tile([128, 128], dt.bfloat16, name="lh")
                    A.mul(lh_t[:], eys[t][:], u[:, t:t + 1])
                    tiles.append((lh_t, rhs_t))
                    PE.matmul(S[1][:], lh_t[:], rhs_t[:, 512:1024],
                              start=(t == 0), stop=(t == NT - 1))
                A.add_instruction(mybir.InstLoadActFuncSet(
                    name=nc.get_next_instruction_name(),
                    act_func_set_id=LNEXP_SET_ID))
                for t in range(NT):
                    V.tensor_scalar(tiles[t][1][:, 0:512], iota1024[:, 0:512],
                                    cstar[:, t:t + 1], None, Alu.is_equal)
                    PE.matmul(S[0][:], tiles[t][0][:], tiles[t][1][:, 0:512],
                              start=(t == 0), stop=(t == NT - 1))

                # ---- per-half decode + split DT chains ----
                HALVES = [(0, [9, 8, 7, 6]), (4, [5, 4, 3, 2])]
                PLANE_ENG_X = [{1: V, 2: V, 3: A, 4: A, 5: A, 6: G, 7: G,
                                8: G, 9: G},
                               {1: V, 2: A, 3: A, 4: G, 5: G}]
                PLANE_ENG_Y = [{1: V, 2: A, 3: A, 4: G, 5: G, 6: G, 7: G,
                                8: G, 9: G},
                               {1: V, 2: A, 3: G, 4: G, 5: G}]
                Sb = wpool.tile([128, 1024], dt.float32, name="Sb")
                Ln2 = wpool.tile([128, 1024], dt.float16, name="Ln2")
                L = wpool.tile([128, 1024], dt.float16, name="L")
                Tp = psT.tile([128, 1024], dt.float16, name="Tp")
                L2 = wpool.tile([128, 1024], dt.float16, name="L2")
                H = wpool.tile([128, 1024], dt.float16, name="H")
                acc2 = [None, None]

                def decode(h):
                    o = h * 512
                    V.tensor_scalar(Sb[:, o:o + 512], S[h][:],
                                    _f(LN_BIAS), None, Alu.max)
                    A.activation(Ln2[:, o:o + 512],
                                 Sb[:, o:o + 512].bitcast(dt.int32), Act.Ln,
                                 scale=_f(2.0 ** -23 / K_ENC), bias=lnbias[:])
                    j0, bks = HALVES[h]
                    ieng = G if h == 0 else V
                    for j, b in zip(range(j0, j0 + 4), bks):
                        inv_s = _f((2 * b + 1) ** 2 / np.float32(18.0))
                        ieng.tensor_scalar(L[:, j * 128:(j + 1) * 128],
                                           Ln2[:, j * 128:(j + 1) * 128],
                                           inv_s, None, Alu.mult)

                def pyramid(h, src, pass_id, src_early=None):
                    """acc copy + bias planes (engines by consumption time).
                    src_early: alternate source (e.g. the PSUM transpose
                    tile) for the V/A planes + acc init, so they don't wait
                    on the SBUF copy of src."""
                    peng = (PLANE_ENG_X if pass_id == 0 else PLANE_ENG_Y)[h]
                    j0, bks = HALVES[h]
                    o = j0 * 128
                    nact = [sum(1 for b in bks if b >= m) for m in range(bks[0] + 1)]
                    se = src if src_early is None else src_early
                    acc = wpool.tile([128, 512], dt.float16,
                                     name=f"acc{h}{pass_id}")
                    V.tensor_copy(acc[:], se[:, o:o + 512])
                    tmps = []
                    for m in range(1, bks[0] + 1):
                        n = nact[m]
                        tm = wpool.tile([128, 512], dt.float16,
                                        name=f"tmp{h}{pass_id}{m}")
                        tmps.append(tm)
                        eng = peng[m]
                        esrc = src if eng is G else se
                        if eng is A:
                            A.activation(
                                xp3(tm[:], [[128, n], [1, 128]]),
                                xp3(esrc[:], [[128, n], [1, 128]], extra_off=o),
                                Act.Copy, bias=-float(m * m))
                        else:
                            eng.tensor_scalar(
                                xp3(tm[:], [[128, n], [1, 128]]),
                                xp3(esrc[:], [[128, n], [1, 128]], extra_off=o),
                                _f(-float(m * m)), None, Alu.add)
                    return acc, tmps, nact

                def chain(h, acc, tmps, nact, on_block_done):
                    bks = HALVES[h][1]
                    for m in range(1, bks[0] + 1):
                        n = nact[m]
                        tm = tmps[m - 1]
                        wlen = 128 - m
                        pv = xp3(acc[:], [[128, n], [1, wlen]], extra_off=m)
                        tv = xp3(tm[:], [[128, n], [1, wlen]])
                        V.tensor_tensor(pv, pv, tv, Alu.max)
                        nv = xp3(acc[:], [[128, n], [1, wlen]])
                        tv2 = xp3(tm[:], [[128, n], [1, wlen]], extra_off=m)
                        V.tensor_tensor(nv, nv, tv2, Alu.max)
                        for jj, b in enumerate(bks):
                            if b == m:
                                on_block_done(jj)
                    return acc

                def xpose_cb(h, acc):
                    j0 = HALVES[h][0]

                    def cb(jj):
                        j = j0 + jj
                        blk = slice(j * 128, (j + 1) * 128)
                        lblk = slice(jj * 128, (jj + 1) * 128)
                        PE.transpose(Tp[:, blk], acc[:, lblk], ident16)
                        A.copy(L2[:, blk], Tp[:, blk])
                    return cb

                def heat_cb(h, acc):
                    j0, bks = HALVES[h]

                    def cb(jj):
                        j = j0 + jj
                        b = bks[jj]
                        s_b = _f(np.float32(18.0) / (2 * b + 1) ** 2)
                        A.activation(H[:, j * 128:(j + 1) * 128],
                                     acc[:, jj * 128:(jj + 1) * 128],
                                     Act.Exp, scale=s_b)
                    return cb

                decode(1)
                aB, tB, nB = pyramid(1, L, 0)
                decode(0)
                chain(1, aB, tB, nB, xpose_cb(1, aB))
                aA, tA, nA = pyramid(0, L, 0)
                chain(0, aA, tA, nA, xpose_cb(0, aA))
                aBy, tBy, nBy = pyramid(1, L2, 1)
                chain(1, aBy, tBy, nBy, heat_cb(1, aBy))
                aAy, tAy, nAy = pyramid(0, L2, 1)
                chain(0, aAy, tAy, nAy, heat_cb(0, aAy))
                nc.sync.dma_start(hm_d[:, 512:1024], H[:, 512:1024])
                nc.sync.dma_start(hm_d[:, 0:512], H[:, 0:512])

    nc.compile()
    return nc


def _consts():
    iota1024 = np.arange(1024, dtype=np.float16)
    cst = np.concatenate([
        np.broadcast_to(iota1024, (128, 1024)),
        np.eye(128, dtype=np.float16),
    ], axis=1)
    return np.ascontiguousarray(cst)


def _shard_inputs(refined_rois, refined_scores, medium_gts, medium_scores,
                  near_unmatched, medium_unmatched):
    """Build the 8 per-core input maps (pure layout/sharding, no math)."""
    cst = _consts()
    coef = np.concatenate([
        np.broadcast_to(np.repeat(np.float32(k), NT), (128, 3 * NT))
        if False else
        np.broadcast_to(np.repeat(np.asarray(k, np.float32), NT), (128, 3 * NT))
        for k in (R_ALPHA, R_BETA, R_GAMMA)
    ], axis=1).astype(np.float32)
    in_maps = []
    B = refined_rois.shape[0]
    n_rr = refined_rois.shape[1]
    n_nu = near_unmatched.shape[1]
    n_mu = medium_unmatched.shape[1]
    for f in range(B):
        bx = np.concatenate([refined_rois[f][:, :7], medium_gts[f][:, :7],
                             near_unmatched[f][:, :7], medium_unmatched[f][:, :7]], 0)
        score = np.concatenate([refined_scores[f], medium_scores[f],
                                np.full(n_nu, 0.4, np.float32),
                                np.full(n_mu, 0.2, np.float32)])
        cls = np.concatenate([np.full(n_rr, -1.0, np.float32), medium_gts[f][:, 7],
                              np.full(n_nu, -1.0, np.float32),
                              np.full(n_mu, -1.0, np.float32)])
        for hf in range(2):
            sl = slice(hf * NBOX, (hf + 1) * NBOX)

            def lay(a):
                return a[sl].astype(np.float32).reshape(NT, 128).T

            par = np.concatenate([lay(bx[:, 0]), lay(bx[:, 1]), lay(bx[:, 3]),
                                  lay(bx[:, 4]), lay(score), lay(cls), coef],
                                 axis=1)
            in_maps.append(dict(par=np.ascontiguousarray(par), cst=cst))
    return in_maps


def kernel(**inputs) -> np.ndarray:
    from concourse.bass_utils import run_bass_kernel_spmd

    if "nc" not in _prog_cache:
        _prog_cache["nc"] = _build_program()
    nc = _prog_cache["nc"]

    in_maps = _shard_inputs(**{k: np.asarray(v) for k, v in inputs.items()})
    res = run_bass_kernel_spmd(nc, in_maps, core_ids=list(range(8)))
    B = np.asarray(inputs["refined_rois"]).shape[0]
    out = np.empty((B, 1, FEAT, FEAT), np.float32)
    for f in range(B):
        m = None
        for c in (2 * f, 2 * f + 1):
            t = res.results[c]["hm"]
            q = t.reshape(128, 8, 128).max(axis=1)
            m = q if m is None else np.maximum(m, q)
        out[f, 0] = m.astype(np.float32).T
    return out


# revision 31
# speedup vs baseline: 1.0238x; 1.0238x over previous
"""BEV distillation mask generator (CenterPoint-style gaussian max-scatter) on TRN2.

Strategy (8 NeuronCores, data-parallel):
  core c handles frame c//2, box-half c%2 (1280 of 2560 boxes per frame).
  Per core:
    1. per-box params (radius bucket, cell, value) via DVE ops, boxes on
       partitions ([128, 10] tiles); radius via r_i = a_i*b + sqrt(beta_i*b^2
       + gamma_i*wh) with host-sent coefficient rows (one ACT sqrt)
    2. exponential encoding u ~= 2^(K*v) via the inverse float-bits trick
       (bits = (K*v+127)*2^23): f32 PSUM *sum* over colliding boxes
       approximates *max* with error <= log2(n)/K; the data has only 2-way
       collisions -> error <= 1/120 ~ 0.008 << 2e-2 tol.
    3. scatter u to 8 per-bucket point images (descending buckets 9..2,
       block index 9-b) via one-hot matmuls on PE
    4. decode: v-hat = log2(max(S,1.1))/K via the same float-bits map
       (exact inverse for singletons), Ln(v-hat) on ACT, per-bucket
       2*sigma^2 scale -> scaled-log image, fp16
    5. gaussian max-envelope = separable 2-pass shift-max DT in scaled-log
       space (additive -mag^2, bucket-independent, one merged 1024-wide
       chain): bias planes on DVE/ACT/Pool by consumption time, serial
       fp16 tensor_tensor maxes (2x) on DVE; per-block transposes (pass 1)
       and heat exps (pass 2) fire as blocks finish
    6. pairwise bucket max -> [x, 2*128] fp16 out
  Host combines column blocks, box-halves and frames with np.maximum and
  transposes to [y, x] (max-scatter is commutative) -> [4,1,128,128] f32.
"""
import numpy as np

FEAT = 128
NBOX = 1280          # boxes per core (half frame)
NT = NBOX // 128     # 10 box tiles
BUCKETS = [9, 8, 7, 6, 5, 4, 3, 2]   # block j holds bucket 9-j
K_ENC = 120.0
LN_BIAS = 1.1
LNEXP_SET_ID = 6     # act_info.json index of natural_log_exp_and_others
# radius: r_i = alpha_i*b1 + sqrt(beta_i*b1^2 + gamma_i*w*h), b1 = h+w (fm)
R_ALPHA = (0.5, 1.0, -0.1)
R_BETA = (0.25, 1.0, 0.01)
R_GAMMA = (-0.25 * 4.0 * 0.9 / 1.1, -3.6, 0.36)

_prog_cache = {}


def _f(x):
    return float(np.float32(x))


def _build_program():
    import concourse.bass as bass
    import concourse.tile as tile
    from concourse import bacc, mybir

    dt = mybir.dt
    Alu = mybir.AluOpType
    Act = mybir.ActivationFunctionType

    nc = bacc.Bacc("TRN2", target_bir_lowering=False, debug=False, num_devices=8)

    NPAR = 6 * NT + 9 * NT
    par_d = nc.dram_tensor("par", [128, NPAR], dt.float32, kind="ExternalInput").ap()
    cst_d = nc.dram_tensor("cst", [128, 1152], dt.float16, kind="ExternalInput").ap()
    hm_d = nc.dram_tensor("hm", [128, 1024], dt.float16, kind="ExternalOutput").ap()

    RECIP08 = _f(1.0 / np.float64(np.float32(0.8)))
    MAGIC = _f(8388608.0)

    def xp3(ap, dims, extra_off=0):
        return type(ap)(ap.tensor, ap.offset + extra_off, [ap.ap[0]] + dims)

    with tile.TileContext(nc) as tc:
        with (
            tc.tile_pool(name="const", bufs=1) as cpool,
            tc.tile_pool(name="par", bufs=1) as ppool,
            tc.tile_pool(name="mm", bufs=NT) as mpool,
            tc.tile_pool(name="dtw", bufs=1) as wpool,
        ):
            cst = cpool.tile([128, 1152], dt.float16, name="cst")
            nc.sync.dma_start(cst[:, 1024:1152], cst_d[:, 1024:1152])
            par = ppool.tile([128, NPAR], dt.float32, name="par")
            nc.sync.dma_start(par[:], par_d)
            nc.sync.dma_start(cst[:, 0:1024], cst_d[:, 0:1024])
            x = par[:, 0:NT]
            y = par[:, NT:2 * NT]
            wl = par[:, 2 * NT:4 * NT]
            sc = par[:, 4 * NT:5 * NT]
            cl = par[:, 5 * NT:6 * NT]
            arow = par[:, 6 * NT:9 * NT]
            brow = par[:, 9 * NT:12 * NT]
            grow = par[:, 12 * NT:15 * NT]
            iota1024 = cst[:, 0:1024]
            iota128 = cst[:, 0:128]
            ident16 = cst[:, 1024:1152]

            V = nc.vector    # DVE
            A = nc.scalar    # ACT
            G = nc.gpsimd    # Pool
            PE = nc.tensor

            _ptn = [0]

            def pt(shape=(128, NT), dtt=None, pool=ppool):
                _ptn[0] += 1
                return pool.tile(list(shape), dtt or dt.float32, name=f"pt{_ptn[0]}")

            def floor_(dst, src_ap, scr):
                V.tensor_scalar(dst, src_ap, MAGIC, MAGIC, Alu.add, Alu.subtract)
                V.tensor_tensor(scr, dst, src_ap, Alu.is_gt)
                V.tensor_tensor(dst, dst, scr, Alu.subtract)

            def b3(t):   # broadcast a [128, NT] view over the 3 radius formulas
                return xp3(t, [[0, 3], [1, NT]])

            # ---- radius (part 1: up to the sqrt argument) ----
            wlf = pt((128, 2 * NT))
            V.tensor_scalar(wlf[:], wl, RECIP08, None, Alu.mult)
            w_fm, l_fm = wlf[:, 0:NT], wlf[:, NT:2 * NT]
            b1 = pt()
            V.tensor_tensor(b1[:], l_fm, w_fm, Alu.add)
            twh = pt()
            V.tensor_tensor(twh[:], w_fm, l_fm, Alu.mult)
            b1sq = pt()
            V.tensor_tensor(b1sq[:], b1[:], b1[:], Alu.mult)
            D3 = pt((128, 3 * NT))
            V.tensor_tensor(D3[:], b3(b1sq[:]), brow, Alu.mult)
            G3 = pt((128, 3 * NT))
            V.tensor_tensor(G3[:], b3(twh[:]), grow, Alu.mult)
            V.tensor_tensor(D3[:], D3[:], G3[:], Alu.add)
            A.activation(D3[:], D3[:], Act.Sqrt)

            # ---- value + encode (fills the sqrt round-trip) ----
            ne7 = pt(); V.tensor_scalar(ne7[:], cl, _f(7.0), None, Alu.not_equal)
            sm = pt(); V.scalar_tensor_tensor(sm[:], cl, _f(5.0), ne7[:], Alu.is_ge, Alu.mult)
            vm = pt(); V.tensor_tensor(vm[:], sm[:], sc, Alu.mult)
            V.tensor_scalar(vm[:], vm[:], _f(0.5), _f(0.5), Alu.mult, Alu.add)
            mgf = pt(); V.tensor_scalar(mgf[:], cl, _f(0.0), None, Alu.is_ge)
            dv = pt()
            V.tensor_tensor(dv[:], vm[:], sc, Alu.subtract)
            V.tensor_tensor(dv[:], dv[:], mgf[:], Alu.mult)
            v = pt()
            V.tensor_tensor(v[:], sc, dv[:], Alu.add)
            qf = pt()
            V.tensor_scalar(qf[:], v[:], _f(K_ENC * 8388608.0),
                            _f(127.0 * 8388608.0), Alu.mult, Alu.add)
            qi = pt(dtt=dt.int32)
            V.tensor_copy(qi[:], qf[:])
            u = qi[:].bitcast(dt.float32)

            # ---- centers; Pool builds the y one-hots off cy ----
            txv = pt(); V.tensor_scalar(txv[:], x, _f(51.2), RECIP08, Alu.add, Alu.mult)
            tyv = pt(); V.tensor_scalar(tyv[:], y, _f(51.2), RECIP08, Alu.add, Alu.mult)
            fscr = pt()
            cx = pt(); floor_(cx[:], txv[:], fscr[:])
            cy = pt(); floor_(cy[:], tyv[:], fscr[:])
            eys = []
            for t in range(NT):
                ey_t = mpool.tile([128, 128], dt.bfloat16, name="ey")
                G.tensor_scalar(ey_t[:], iota128, cy[:, t:t + 1], None,
                                Alu.is_equal)
                eys.append(ey_t)

            # ---- radius part 2 + bucket code ----
            R3 = pt((128, 3 * NT))
            V.tensor_tensor(R3[:], b3(b1[:]), arow, Alu.mult)
            V.tensor_tensor(R3[:], R3[:], D3[:], Alu.add)
            r = pt()
            V.tensor_tensor(r[:], R3[:, 0:NT], R3[:, NT:2 * NT], Alu.min)
            V.tensor_tensor(r[:], r[:], R3[:, 2 * NT:3 * NT], Alu.min)
            fb = pt(); floor_(fb[:], r[:], fscr[:])
            jc = pt()
            V.tensor_scalar(jc[:], fb[:], _f(2.0), _f(9.0), Alu.max, Alu.min)
            V.tensor_scalar(jc[:], jc[:], _f(-128.0), _f(1152.0), Alu.mult, Alu.add)
            cstar = pt()
            V.tensor_tensor(cstar[:], jc[:], cx[:], Alu.add)

            lnbias = cpool.tile([128, 1], dt.float32, name="lnbias")
            V.memset(lnbias[:], _f(-127.0 / K_ENC))

            # ---- one-hots + scatter matmuls ----
            with (
                tc.tile_pool(name="psS", bufs=2, space="PSUM") as psS,
                tc.tile_pool(name="psT", bufs=1, space="PSUM") as psT,
            ):
                S = [psS.tile([128, 512], dt.float32, name=f"S{h}") for h in range(2)]
                # p-state warmup: keep PE busy from early on so the real
                # matmul stream runs at full clock
                trash = psT.tile([128, 128], dt.float16, name="trash")
                for _ in range(17):
                    PE.transpose(trash[:], ident16, ident16)
                tiles = []
                for t in range(NT):
                    rhs_t = mpool.tile([128, 1024], dt.float16, name="rhs")
                    V.tensor_scalar(rhs_t[:, 512:1024], iota1024[:, 512:1024],
                                    cstar[:, t:t + 1], None, Alu.is_equal)
                    lh_t = mpool.tile([128, 128], dt.bfloat16, name="lh")
                    A.mul(lh_t[:], eys[t][:], u[:, t:t + 1])
                    tiles.append((lh_t, rhs_t))
                    PE.matmul(S[1][:], lh_t[:], rhs_t[:, 512:1024],
                              start=(t == 0), stop=(t == NT - 1))
                A.add_instruction(mybir.InstLoadActFuncSet(
                    name=nc.get_next_instruction_name(),
                    act_func_set_id=LNEXP_SET_ID))
                for t in range(NT):
                    V.tensor_scalar(tiles[t][1][:, 0:512], iota1024[:, 0:512],
                                    cstar[:, t:t + 1], None, Alu.is_equal)
                    PE.matmul(S[0][:], tiles[t][0][:], tiles[t][1][:, 0:512],
                              start=(t == 0), stop=(t == NT - 1))

                # ---- per-half decode + split DT chains ----
                HALVES = [(0, [9, 8, 7, 6]), (4, [5, 4, 3, 2])]
                PLANE_ENG_X = [{1: V, 2: V, 3: A, 4: A, 5: A, 6: G, 7: G,
                                8: G, 9: G},
                               {1: V, 2: A, 3: A, 4: G, 5: G}]
                PLANE_ENG_Y = [{1: V, 2: A, 3: A, 4: G, 5: G, 6: G, 7: G,
                                8: G, 9: G},
                               {1: V, 2: A, 3: G, 4: G, 5: G}]
                Sb = wpool.tile([128, 1024], dt.float32, name="Sb")
                Ln2 = wpool.tile([128, 1024], dt.float16, name="Ln2")
                L = wpool.tile([128, 1024], dt.float16, name="L")
                Tp = psT.tile([128, 1024], dt.float16, name="Tp")
                L2 = wpool.tile([128, 1024], dt.float16, name="L2")
                H = wpool.tile([128, 1024], dt.float16, name="H")
                acc2 = [None, None]

                def decode(h):
                    o = h * 512
                    V.tensor_scalar(Sb[:, o:o + 512], S[h][:],
                                    _f(LN_BIAS), None, Alu.max)
                    A.activation(Ln2[:, o:o + 512],
                                 Sb[:, o:o + 512].bitcast(dt.int32), Act.Ln,
                                 scale=_f(2.0 ** -23 / K_ENC), bias=lnbias[:])
                    j0, bks = HALVES[h]
                    ieng = G if h == 0 else V
                    for j, b in zip(range(j0, j0 + 4), bks):
                        inv_s = _f((2 * b + 1) ** 2 / np.float32(18.0))
                        ieng.tensor_scalar(L[:, j * 128:(j + 1) * 128],
                                           Ln2[:, j * 128:(j + 1) * 128],
                                           inv_s, None, Alu.mult)

                def pyramid(h, src, pass_id, src_early=None):
                    """acc copy + bias planes (engines by consumption time).
                    src_early: alternate source (e.g. the PSUM transpose
                    tile) for the V/A planes + acc init, so they don't wait
                    on the SBUF copy of src."""
                    peng = (PLANE_ENG_X if pass_id == 0 else PLANE_ENG_Y)[h]
                    j0, bks = HALVES[h]
                    o = j0 * 128
                    nact = [sum(1 for b in bks if b >= m) for m in range(bks[0] + 1)]
                    se = src if src_early is None else src_early
                    acc = wpool.tile([128, 512], dt.float16,
                                     name=f"acc{h}{pass_id}")
                    V.tensor_copy(acc[:], se[:, o:o + 512])
                    tmps = []
                    for m in range(1, bks[0] + 1):
                        n = nact[m]
                        tm = wpool.tile([128, 512], dt.float16,
                                        name=f"tmp{h}{pass_id}{m}")
                        tmps.append(tm)
                        eng = peng[m]
                        esrc = src if eng is G else se
                        if eng is A:
                            A.activation(
                                xp3(tm[:], [[128, n], [1, 128]]),
                                xp3(esrc[:], [[128, n], [1, 128]], extra_off=o),
                                Act.Copy, bias=-float(m * m))
                        else:
                            eng.tensor_scalar(
                                xp3(tm[:], [[128, n], [1, 128]]),
                                xp3(esrc[:], [[128, n], [1, 128]], extra_off=o),
                                _f(-float(m * m)), None, Alu.add)
                    return acc, tmps, nact

                def chain(h, acc, tmps, nact, on_block_done):
                    bks = HALVES[h][1]
                    for m in range(1, bks[0] + 1):
                        n = nact[m]
                        tm = tmps[m - 1]
                        wlen = 128 - m
                        pv = xp3(acc[:], [[128, n], [1, wlen]], extra_off=m)
                        tv = xp3(tm[:], [[128, n], [1, wlen]])
                        V.tensor_tensor(pv, pv, tv, Alu.max)
                        nv = xp3(acc[:], [[128, n], [1, wlen]])
                        tv2 = xp3(tm[:], [[128, n], [1, wlen]], extra_off=m)
                        V.tensor_tensor(nv, nv, tv2, Alu.max)
                        for jj, b in enumerate(bks):
                            if b == m:
                                on_block_done(jj)
                    return acc

                def xpose_cb(h, acc):
                    j0 = HALVES[h][0]

                    def cb(jj):
                        j = j0 + jj
                        blk = slice(j * 128, (j + 1) * 128)
                        lblk = slice(jj * 128, (jj + 1) * 128)
                        PE.transpose(Tp[:, blk], acc[:, lblk], ident16)
                        A.copy(L2[:, blk], Tp[:, blk])
                    return cb

                def heat_cb(h, acc):
                    j0, bks = HALVES[h]

                    def cb(jj):
                        j = j0 + jj
                        b = bks[jj]
                        s_b = _f(np.float32(18.0) / (2 * b + 1) ** 2)
                        A.activation(H[:, j * 128:(j + 1) * 128],
                                     acc[:, jj * 128:(jj + 1) * 128],
                                     Act.Exp, scale=s_b)
                    return cb

                decode(1)
                aB, tB, nB = pyramid(1, L, 0)
                decode(0)
                chain(1, aB, tB, nB, xpose_cb(1, aB))
                aA, tA, nA = pyramid(0, L, 0)
                chain(0, aA, tA, nA, xpose_cb(0, aA))
                aBy, tBy, nBy = pyramid(1, L2, 1)
                chain(1, aBy, tBy, nBy, heat_cb(1, aBy))
                aAy, tAy, nAy = pyramid(0, L2, 1)
                chain(0, aAy, tAy, nAy, heat_cb(0, aAy))
                nc.sync.dma_start(hm_d[:, 512:1024], H[:, 512:1024])
                nc.sync.dma_start(hm_d[:, 0:512], H[:, 0:512])

    nc.compile()
    return nc


def _consts():
    iota1024 = np.arange(1024, dtype=np.float16)
    cst = np.concatenate([
        np.broadcast_to(iota1024, (128, 1024)),
        np.eye(128, dtype=np.float16),
    ], axis=1)
    return np.ascontiguousarray(cst)


def _shard_inputs(refined_rois, refined_scores, medium_gts, medium_scores,
                  near_unmatched, medium_unmatched):
    """Build the 8 per-core input maps (pure layout/sharding, no math)."""
    cst = _consts()
    coef = np.concatenate([
        np.broadcast_to(np.repeat(np.float32(k), NT), (128, 3 * NT))
        if False else
        np.broadcast_to(np.repeat(np.asarray(k, np.float32), NT), (128, 3 * NT))
        for k in (R_ALPHA, R_BETA, R_GAMMA)
    ], axis=1).astype(np.float32)
    in_maps = []
    B = refined_rois.shape[0]
    n_rr = refined_rois.shape[1]
    n_nu = near_unmatched.shape[1]
    n_mu = medium_unmatched.shape[1]
    for f in range(B):
        bx = np.concatenate([refined_rois[f][:, :7], medium_gts[f][:, :7],
                             near_unmatched[f][:, :7], medium_unmatched[f][:, :7]], 0)
        score = np.concatenate([refined_scores[f], medium_scores[f],
                                np.full(n_nu, 0.4, np.float32),
                                np.full(n_mu, 0.2, np.float32)])
        cls = np.concatenate([np.full(n_rr, -1.0, np.float32), medium_gts[f][:, 7],
                              np.full(n_nu, -1.0, np.float32),
                              np.full(n_mu, -1.0, np.float32)])
        for hf in range(2):
            sl = slice(hf * NBOX, (hf + 1) * NBOX)

            def lay(a):
                return a[sl].astype(np.float32).reshape(NT, 128).T

            par = np.concatenate([lay(bx[:, 0]), lay(bx[:, 1]), lay(bx[:, 3]),
                                  lay(bx[:, 4]), lay(score), lay(cls), coef],
                                 axis=1)
            in_maps.append(dict(par=np.ascontiguousarray(par), cst=cst))
    return in_maps


def kernel(**inputs) -> np.ndarray:
    from concourse.bass_utils import run_bass_kernel_spmd

    if "nc" not in _prog_cache:
        _prog_cache["nc"] = _build_program()
    nc = _prog_cache["nc"]

    in_maps = _shard_inputs(**{k: np.asarray(v) for k, v in inputs.items()})
    res = run_bass_kernel_spmd(nc, in_maps, core_ids=list(range(8)))
    B = np.asarray(inputs["refined_rois"]).shape[0]
    out = np.empty((B, 1, FEAT, FEAT), np.float32)
    for f in range(B):
        m = None
        for c in (2 * f, 2 * f + 1):
            t = res.results[c]["hm"]
            q = t.reshape(128, 8, 128).max(axis=1)
            m = q if m is None else np.maximum(m, q)
        out[f, 0] = m.astype(np.float32).T
    return out
